# revision 7
# baseline (speedup 1.0000x reference)
"""Multi-head causal attention (B=2, L=2048, D=1024, H=16, Hd=64) on 8 TRN2
NeuronCores.

Sharding: data-parallel over the 2 batches x tensor-parallel over heads
(4 cores per batch, 4 heads per core).  Each core computes its heads'
QKV projection, attention, and a partial out-projection over its 256
local dims; the host sums the 4 partials per batch.

All matmul operands are fp16 (full-rate PE streaming + FWL weight load,
half the HBM traffic); accumulation stays fp32 in PSUM.

Per-core dataflow (per head pair hp, bases 0/64 of the m-tiles):
  qT,kT  [512, L]  = wqkT.T @ xT           (scale 1/8 folded into wq)
  v      [L, 256]  = xT.T-tiles @ wvT      ([l,d] layout)
  S^T    [128k, 512q] = kT_h.T @ qT_h      (K=64, the pair's two heads on
         disjoint PE row groups run concurrently)
  E      = exp(S^T)                        (one ACT op per k-tile, both heads)
  av     [128, 512q] += v_h.T @ E_h        (M=64 col-tiled: the pair's two
         AV matmuls run concurrently on disjoint PE column groups)
  den    [4x1, 512q] += 1.T @ E            (4-way col-tiled M=1 matmuls --
         two k-tiles x two heads per pass -- every other k-tile)
  attnT  = av * reciprocal(den0 + den1)    (DVE fold + recip + gpsimd
         partition_broadcast + DVE mul straight out of PSUM)
  out    [L, 1024] += attnT-pair.T @ woT-pair    (K=128 per head pair)

Compared to the M=65 [v|1] ones-column trick, the col-tiled AV + packed
denominator pass cuts AV streaming cycles from 2x to 1.25x the S^T
cycles (the M=65 form wastes half the PE columns).

Scheduling: the ACT exp chain (~83us) and the PE stream (~87us) are both
near the kernel's critical path, so emission is paced by a greedy weave
that tracks estimated PE/ACT clocks: attention S^T steps are emitted as
late as the exp backlog allows, with projection / out-projection groups
(pure PE work) pulled from a requirement-tagged filler queue in between.
exp starts ~14us in (right after the first QKV group's dependencies
land) instead of after two full QKV chunks.  Input DMAs issue on the
sync/vector/gpsimd rings, critical tiles first; the scalar queue is kept
free for the exp stream.  One shared 8-bank PSUM pool: qkps 1 + vps 1 +
st 2x2 + av 1 + den 1 = 8; the out-projection reuses qkps/vps (and idle
st banks at the tail).
"""
import sys
sys.path.insert(0, '/opt/trn_rl_repo')
import numpy as np

B, L, D = 2, 2048, 1024
H, HD = 16, 64
NCORES = 8
CPB = 4              # cores per batch
HPC = H // CPB       # heads per core = 4
DLOC = HPC * HD      # 256 local head dims per core
NKT, NQT = L // 128, L // 512   # 16 k-tiles, 4 q-tiles
NEG = -30000.0

_built = {}


def _build(status, use_cb):
    """status: [NKT, NQT] int8 (0=skip, 1=full, 2=mixed); use_cb: causal
    on-chip bias patterns (True) vs DMA'd bias tiles (False)."""
    import concourse.mybir as mybir
    import concourse.tile as tile
    from concourse import bacc

    F32 = mybir.dt.float32
    F16 = mybir.dt.float16
    Exp = mybir.ActivationFunctionType.Exp

    # mixed-block index map for the DMA'd-bias mode
    mixed_ids = {}
    for qt in range(NQT):
        for kt in range(NKT):
            if status[kt, qt] == 2:
                mixed_ids[(kt, qt)] = len(mixed_ids)
    nmix = len(mixed_ids)

    nc = bacc.Bacc("TRN2", target_bir_lowering=False, debug=False)
    xT_d = nc.dram_tensor("xT", [D, L], F16, kind="ExternalInput")
    wqkT_d = nc.dram_tensor("wqkT", [D, 2 * DLOC], F16, kind="ExternalInput")
    wvT_d = nc.dram_tensor("wvT", [D, DLOC], F16, kind="ExternalInput")
    woT_d = nc.dram_tensor("woT", [128, 2 * D], F16, kind="ExternalInput")
    if not use_cb and nmix:
        bias_d = nc.dram_tensor("bias", [nmix, 128, 512], F32, kind="ExternalInput")
    out_d = nc.dram_tensor("out", [L, D], F16, kind="ExternalOutput")

    with tile.TileContext(nc) as tc:
        with tc.tile_pool(name="const", bufs=1) as const, \
             tc.tile_pool(name="esp", bufs=5) as esp, \
             tc.tile_pool(name="misc", bufs=2) as misc, \
             tc.tile_pool(name="otp", bufs=3) as otp:

            # ---- input loads: sync/vector/gpsimd issue rings (scalar is
            # reserved for the exp stream), critical tiles first: the first
            # attention unit needs wqk groups 0/2 + x quarter 0; wv right
            # behind for the v fillers, then the rest in use order ----
            wqr = wqkT_d.ap().rearrange("(a p) m -> p a m", p=128)
            wqkg = [const.tile([128, D // 128, 128], F16, tag=f"wqk{g}",
                               name=f"wqk{g}") for g in range(4)]
            xqt = [const.tile([128, D // 128, 512], F16, tag=f"xq{q}",
                              name=f"xq{q}") for q in range(4)]
            xr = xT_d.ap().rearrange("(a p) l -> p a l", p=128)
            wv = const.tile([128, D // 128, DLOC], F16, tag="wv")
            wo = const.tile([128, 2 * D], F16, tag="wo")
            ones = const.tile([128, 1], F16, tag="ones")

            nc.sync.dma_start(out=wqkg[0], in_=wqr[:, :, 0:128])
            nc.scalar.dma_start(out=wqkg[2], in_=wqr[:, :, 256:384])
            nc.gpsimd.dma_start(out=xqt[0], in_=xr[:, :, 0:512])
            nc.sync.dma_start(
                out=wv, in_=wvT_d.ap().rearrange("(a p) m -> p a m", p=128))
            nc.scalar.dma_start(out=wqkg[1], in_=wqr[:, :, 128:256])
            nc.sync.dma_start(out=wqkg[3], in_=wqr[:, :, 384:512])
            nc.gpsimd.dma_start(out=xqt[1], in_=xr[:, :, 512:1024])
            nc.scalar.dma_start(out=xqt[2], in_=xr[:, :, 1024:1536])
            nc.sync.dma_start(out=xqt[3], in_=xr[:, :, 1536:2048])
            nc.gpsimd.dma_start(out=wo, in_=woT_d.ap())
            nc.vector.memset(ones, 1.0)

            def xslice(l0, l1):
                q = l0 // 512
                assert l1 <= (q + 1) * 512
                return lambda k: xqt[q][:, k, l0 - q * 512:l1 - q * 512]

            # ---- causal 0/1 mask patterns (r = kt - 4*qt in 0..3) ----
            if use_cb:
                cb = const.tile([128, 4, 512], F16, tag="cb")
                nc.vector.memset(cb, 1.0)
                for r in range(4):
                    # keep 1.0 where -k + q - 128r >= 0 (attend), else 0.0
                    nc.gpsimd.affine_select(
                        out=cb[:, r, :],
                        in_=cb[:, r, :],
                        compare_op=mybir.AluOpType.is_ge, fill=0.0,
                        base=-128 * r, channel_multiplier=-1,
                        pattern=[[1, 512]])

            # per-L-tile projection results
            qkl = [const.tile([128, 4, 512], F16, tag=f"qk{lt}",
                              name=f"qk{lt}")
                   for lt in range(NQT)]
            vtg = [const.tile([128, 4, DLOC], F16, tag=f"vt{g}",
                              name=f"vt{g}")
                   for g in range(NQT)]

            with tc.tile_pool(name="psum", bufs=1, space="PSUM") as psum, \
                 tc.tile_pool(name="atp", bufs=4) as atp:

                def qk_group(lt, g):
                    ps = psum.tile([128, 512], F32, tag="qkps", bufs=1,
                                   name=f"qkps{lt}{g}")
                    xs = xslice(lt * 512, (lt + 1) * 512)
                    for kt in range(D // 128):
                        nc.tensor.matmul(
                            ps, wqkg[g][:, kt, :], xs(kt),
                            start=(kt == 0), stop=(kt == D // 128 - 1))
                    nc.vector.tensor_copy(qkl[lt][:, g, :], ps)

                def v_group(lt, g):
                    l16 = 4 * lt + g
                    psv = psum.tile([128, DLOC], F32, tag="vps", bufs=1,
                                    name=f"vps{l16}")
                    xs = xslice(l16 * 128, (l16 + 1) * 128)
                    for kt in range(D // 128):
                        nc.tensor.matmul(
                            psv, xs(kt), wv[:, kt, :],
                            start=(kt == 0), stop=(kt == D // 128 - 1))
                    nc.vector.tensor_copy(vtg[lt][:, g, :], psv)

                # ---- attention unit (one q-tile x one head pair) ----
                # Steps: S(i) = S^T pair matmuls + exp; A(i) = col-tiled AV
                # pair; D(i2) = 4-way packed denominator pass over k-tile
                # indices 2*i2, 2*i2+1; norm = fold + recip + bcast + mul.
                at_tiles = {}

                def unit_steps(qt, hp, extra_prenorm=None):
                    kts = [kt for kt in range(NKT) if status[kt, qt] != 0]
                    n = len(kts)
                    cell = {}
                    if qt not in at_tiles:
                        at_tiles[qt] = [
                            atp.tile([128, 512], F16, tag=f"at{p}",
                                     name=f"at{p}_{qt}") for p in range(2)]
                    mq, mk = hp, 2 + hp

                    def s_step(i):
                        kt = kts[i]
                        if i == 0:
                            cell["av"] = psum.tile(
                                [128, 512], F32, tag="av", bufs=1,
                                name=f"av{qt}{hp}")
                            cell["den"] = psum.tile(
                                [97, 512], F32, tag="den", bufs=1,
                                name=f"den{qt}{hp}")
                        mixed = status[kt, qt] == 2
                        c0 = 128 * (kt - 4 * qt) if (mixed and use_cb) else 0
                        cell[("c0", i)] = c0
                        st = psum.tile([128, 2, 512], F32, tag="st",
                                       bufs=2, name=f"st{qt}{hp}{kt}")
                        for j, base in ((0, 0), (1, 64)):
                            nc.tensor.matmul(
                                st[:, j, c0:],
                                qkl[kt // 4][base:base + 64, mk,
                                             (kt % 4) * 128:
                                             (kt % 4 + 1) * 128],
                                qkl[qt][base:base + 64, mq, c0:],
                                start=True, stop=True)
                        if mixed and not use_cb:
                            b_ap = misc.tile([128, 512], F32, tag="bt")
                            nc.sync.dma_start(
                                out=b_ap,
                                in_=bias_d.ap()[mixed_ids[(kt, qt)]])
                            for j in range(2):
                                nc.vector.tensor_add(
                                    st[:, j, :], st[:, j, :], b_ap)
                        es = esp.tile([128, 2, 512], F16, tag="es")
                        cell[("es", i)] = es
                        nc.scalar.activation(es[:, :, c0:],
                                             st[:, :, c0:], Exp)
                        if mixed and use_cb:
                            # only the 128-wide diagonal strip is partial
                            nc.vector.tensor_mul(
                                es[:, :, c0:c0 + 128],
                                es[:, :, c0:c0 + 128],
                                cb[:, 0:1, 0:128].broadcast_to(
                                    [128, 2, 128]))

                    def a_step(i):
                        kt = kts[i]
                        c0 = cell[("c0", i)]
                        es = cell[("es", i)]
                        av = cell["av"]
                        for j in (0, 1):
                            h = 2 * hp + j
                            nc.tensor.matmul(
                                av[64 * j:64 * j + 64, c0:],
                                vtg[kt // 4][:, kt % 4,
                                             h * HD:(h + 1) * HD],
                                es[:, j, c0:],
                                start=(i == 0), stop=(i == n - 1),
                                skip_group_check=True)

                    nd = (n + 1) // 2

                    def d_step(i2):
                        den = cell["den"]
                        streams = []
                        for ii in (2 * i2, 2 * i2 + 1):
                            if ii < n:
                                for j in (0, 1):
                                    streams.append((ii, j))
                        for idx, (ii, j) in enumerate(streams):
                            es = cell[("es", ii)]
                            c0 = cell[("c0", ii)]
                            row = 32 * (2 * (ii % 2) + j)
                            nc.tensor.matmul(
                                den[row:row + 1, c0:], ones,
                                es[:, j, c0:],
                                start=(i2 == 0), stop=(i2 == nd - 1),
                                skip_group_check=True,
                                tile_position=(0, row))

                    def norm():
                        av, den = cell["av"], cell["den"]
                        dsb = misc.tile([1, 2, 512], F32, tag="dsb", bufs=2)
                        # rows 0/32 hold even-k sums (always valid from col
                        # 0: the first k-tile is full width); rows 64/96
                        # hold odd-k sums, valid from vo.  Engines can read
                        # only one PSUM operand, so the odd rows are staged
                        # through SBUF (on gpsimd, which is mostly idle).
                        vo = 0
                        if n < 2:
                            vo = 512          # no odd contributions at all
                        elif use_cb and qt == 0:
                            vo = 128          # odd k-tiles are strips
                        so = misc.tile([1, 2, 512], F32, tag="sodd", bufs=2)
                        if vo < 512:
                            nc.vector.tensor_copy(so[:, 0, vo:],
                                                  den[64:65, vo:])
                            nc.vector.tensor_copy(so[:, 1, vo:],
                                                  den[96:97, vo:])
                        for j, r0 in ((0, 0), (1, 32)):
                            if vo > 0:
                                nc.vector.tensor_copy(
                                    dsb[:, j, 0:vo], den[r0:r0 + 1, 0:vo])
                            if vo < 512:
                                nc.vector.tensor_add(
                                    dsb[:, j, vo:], den[r0:r0 + 1, vo:],
                                    so[:, j, vo:])
                        rc = misc.tile([1, 2, 512], F32, tag="rc", bufs=2)
                        nc.vector.reciprocal_approx_fast(out=rc, in_=dsb)
                        bc = misc.tile([64, 2, 512], F32, tag="bc", bufs=2)
                        nc.gpsimd.partition_broadcast(bc, rc, channels=64)
                        for j in (0, 1):
                            nc.vector.tensor_mul(
                                at_tiles[qt][hp][64 * j:64 * j + 64, :],
                                av[64 * j:64 * j + 64, :], bc[:, j, :])

                    # step list: (emit_fn, pe_ns, act_ns, requirements)
                    # A(i) trails S(i+1); D(i2) after A(2*i2+1)
                    steps = []

                    def w(i):
                        return 512 - (cell.get(("c0", i)) or 0)

                    def est_w(i):
                        kt = kts[i]
                        mixed = status[kt, qt] == 2
                        return 512 - (128 * (kt - 4 * qt)
                                      if (mixed and use_cb) else 0)

                    for i in range(n):
                        kt = kts[i]
                        wd = est_w(i)
                        steps.append((
                            (lambda i=i: s_step(i)),
                            wd / 2.4,
                            (2 * wd + 352) / 1.2,
                            [("qk", qt, mq), ("qk", kt // 4, mk)]))
                        if i >= 1:
                            kp = kts[i - 1]
                            steps.append((
                                (lambda i=i - 1: a_step(i)),
                                est_w(i - 1) / 2.4, 0.0,
                                [("v", kp // 4, kp % 4)]))
                            if i % 2 == 1 and i >= 3:
                                steps.append((
                                    (lambda i2=(i - 3) // 2: d_step(i2)),
                                    est_w(i - 3) / 2.4, 0.0, []))
                    kp = kts[n - 1]
                    steps.append(((lambda: a_step(n - 1)),
                                  est_w(n - 1) / 2.4, 0.0,
                                  [("v", kp // 4, kp % 4)]))
                    for i2 in range((nd - 1) if n >= 4 else 0, nd):
                        steps.append(((lambda i2=i2: d_step(i2)),
                                      est_w(min(2 * i2, n - 1)) / 2.4,
                                      0.0, []))
                    if extra_prenorm:
                        steps.append(extra_prenorm)
                    steps.append((norm, 0.0, 0.0, []))
                    return steps

                # ---- out-projection ----
                op3_pos = {}

                def op_lt_pass0(qt, lt, use_st=False):
                    if use_st:
                        t = psum.tile([128, 2, 512], F32, tag="st", bufs=2,
                                      name=f"post{qt}{lt}")
                        pos = [t[:, 0, :], t[:, 1, :]]
                    else:
                        pos = [psum.tile([128, 512], F32, tag=t, bufs=1,
                                         name=f"po{qt}{lt}{t}")
                               for t in ("qkps", "vps")]
                    op3_pos[(qt, lt)] = pos
                    for do in range(2):
                        nc.tensor.matmul(
                            pos[do],
                            at_tiles[qt][0][:, lt * 128:(lt + 1) * 128],
                            wo[:, do * 512:do * 512 + 512],
                            start=True, stop=False)

                def op_lt_pass1(qt, lt, tail=False):
                    pos = op3_pos.pop((qt, lt))
                    row = qt * 512 + lt * 128
                    ot = otp.tile([128, 2, 512], F16, tag="ot")
                    for do in range(2):
                        nc.tensor.matmul(
                            pos[do],
                            at_tiles[qt][1][:, lt * 128:(lt + 1) * 128],
                            wo[:, D + do * 512:D + do * 512 + 512],
                            start=False, stop=True)
                        if tail and do == 1:
                            # tail: ACT is idle; split the evacuation
                            # copies across ACT and DVE
                            nc.scalar.copy(ot[:, do, :], pos[do])
                        else:
                            nc.vector.tensor_copy(ot[:, do, :], pos[do])
                    if tail:
                        eng = nc.scalar if lt % 2 == 1 else nc.sync
                    else:
                        eng = nc.gpsimd if lt % 2 == 1 else nc.sync
                    eng.dma_start(
                        out=out_d.ap()[row:row + 128, :],
                        in_=ot.rearrange("p a b -> p (a b)"))

                def op_lt_step(qt, lt, tail=False):
                    op_lt_pass0(qt, lt, use_st=tail and lt % 2 == 1)
                    op_lt_pass1(qt, lt, tail)

                if use_cb:
                    # ---- greedy paced emission ----
                    # filler queue: (name, fn, pe_ns, req, min_ns)
                    fq = []
                    for lt in range(NQT):
                        arr = 11000.0 + 3500.0 * lt
                        for g in (0, 2, 1, 3):
                            fq.append((("qk", lt, g),
                                       (lambda lt=lt, g=g: qk_group(lt, g)),
                                       1707.0, None, arr))
                        for g in range(4):
                            fq.append((("v", lt, g),
                                       (lambda lt=lt, g=g: v_group(lt, g)),
                                       853.0, None, max(arr, 13000.0)))
                    for qt in range(NQT - 1):
                        for lt in range(4):
                            fq.append((("op", qt, lt),
                                       (lambda qt=qt, lt=lt:
                                        op_lt_step(qt, lt)),
                                       1707.0, ("unit", qt, 1), 22000.0))

                    emitted = set()
                    done_units = set()
                    clk = {"pe": 8300.0, "act": 13500.0}

                    def emit_filler(entry):
                        name, fn, pe_ns, req, min_ns = entry
                        fn()
                        emitted.add(name)
                        clk["pe"] += pe_ns
                        fq.remove(entry)

                    def ensure(name):
                        if name in emitted or name[0] == "unit":
                            return
                        for entry in fq:
                            if entry[0] == name:
                                emit_filler(entry)
                                return

                    def pull_filler():
                        # first ready filler in queue order
                        for entry in fq:
                            req, min_ns = entry[3], entry[4]
                            if req is not None and req not in done_units:
                                continue
                            if min_ns > clk["pe"] + 1500.0:
                                continue
                            return entry
                        return None

                    units = [(qt, hp) for qt in range(NQT) for hp in (0, 1)]
                    for qt, hp in units:
                        extra = None
                        if (qt, hp) == (3, 1):
                            # out-proj(3) pair-0 pass sits directly before
                            # the last normalize, covering its latency
                            extra = ((lambda: op_lt_pass0(3, 0,
                                                          use_st=True)),
                                     427.0, 0.0, [])
                        for fn, pe_ns, act_ns, reqs in unit_steps(
                                qt, hp, extra_prenorm=extra):
                            for r in reqs:
                                ensure(r)
                            if act_ns > 0.0:
                                # S-step: pad with fillers while the exp
                                # backlog can hide them
                                while True:
                                    gap = clk["act"] - (clk["pe"] + pe_ns)
                                    if gap <= 0.0:
                                        break
                                    entry = pull_filler()
                                    if entry is None or entry[2] > gap + 900.0:
                                        break
                                    emit_filler(entry)
                            fn()
                            clk["pe"] += pe_ns
                            if act_ns > 0.0:
                                clk["act"] = max(clk["act"],
                                                 clk["pe"]) + act_ns
                        done_units.add(("unit", qt, hp))

                    # flush leftovers (op(2) usually lands here)
                    while fq:
                        emit_filler(fq[0])
                    op_lt_pass1(3, 0, tail=True)
                    for lt in range(1, 4):
                        op_lt_step(3, lt, tail=True)
                else:
                    for lt in range(NQT):
                        for g in (0, 2, 1, 3):
                            qk_group(lt, g)
                        for g in range(4):
                            v_group(lt, g)
                    for qt in range(NQT):
                        for hp in (0, 1):
                            for fn, _, _, _ in unit_steps(qt, hp):
                                fn()
                        for lt in range(4):
                            op_lt_step(qt, lt, tail=(qt == NQT - 1))
    nc.compile()
    return nc


def _host_prep(x, mask, w_qkv, w_out):
    x = np.asarray(x, dtype=np.float32)
    mask = np.asarray(mask).astype(bool)
    w_qkv = np.asarray(w_qkv, dtype=np.float32)
    w_out = np.asarray(w_out, dtype=np.float32)

    tril = np.tril(np.ones((L, L), dtype=bool))
    is_causal = all(np.array_equal(mask[b], tril) for b in range(B))

    # block classification on the S^T layout: block (kt, qt) covers
    # k in [kt*128, ...), q in [qt*512, ...)
    status = np.zeros((NKT, NQT), np.int8)
    if is_causal:
        for qt in range(NQT):
            for kt in range(NKT):
                r = kt - 4 * qt
                status[kt, qt] = 0 if r > 3 else (2 if r >= 0 else 1)
    else:
        for qt in range(NQT):
            for kt in range(NKT):
                blk = mask[:, qt * 512:(qt + 1) * 512, kt * 128:(kt + 1) * 128]
                status[kt, qt] = 1 if blk.all() else (0 if not blk.any() else 2)

    # per-core inputs
    scale = float(HD) ** -0.5
    in_maps = []
    bias_by_batch = None
    if not is_causal:
        mixed = [(kt, qt) for qt in range(NQT) for kt in range(NKT)
                 if status[kt, qt] == 2]
        if mixed:
            bias_by_batch = []
            for b in range(B):
                tiles = np.zeros((len(mixed), 128, 512), np.float32)
                mt = mask[b].T  # [k, q]
                for i, (kt, qt) in enumerate(mixed):
                    blk = mt[kt * 128:(kt + 1) * 128, qt * 512:(qt + 1) * 512]
                    tiles[i] = np.where(blk, 0.0, NEG)
                bias_by_batch.append(tiles)

    for c in range(NCORES):
        b = c // CPB
        hq = (c % CPB) * HPC
        wq = w_qkv[hq * HD:(hq + HPC) * HD] * scale
        wk = w_qkv[D + hq * HD:D + (hq + HPC) * HD]
        wv = w_qkv[2 * D + hq * HD:2 * D + (hq + HPC) * HD]
        wqkT = np.ascontiguousarray(
            np.concatenate([wq, wk], 0).T.astype(np.float16))
        wvT = np.ascontiguousarray(wv.T.astype(np.float16))
        wo_loc = w_out[:, hq * HD:(hq + HPC) * HD].T       # [256, 1024]
        woT = np.ascontiguousarray(
            wo_loc.reshape(2, 128, D).transpose(1, 0, 2)
            .reshape(128, 2 * D).astype(np.float16))
        im = {"xT": np.ascontiguousarray(x[b].T.astype(np.float16)),
              "wqkT": wqkT, "wvT": wvT, "woT": woT}
        if bias_by_batch is not None:
            im["bias"] = bias_by_batch[b]
        in_maps.append(im)
    return status, is_causal, in_maps


LAST_RESULTS = None


def kernel(x, mask, w_qkv, w_out):
    from concourse.bass_utils import run_bass_kernel_spmd
    global LAST_RESULTS

    status, is_causal, in_maps = _host_prep(x, mask, w_qkv, w_out)
    key = (is_causal, status.tobytes())
    if key not in _built:
        _built[key] = _build(status, is_causal)
    nc = _built[key]

    res = run_bass_kernel_spmd(nc, in_maps, core_ids=list(range(NCORES)))
    LAST_RESULTS = res
    out = np.zeros((B, L, D), np.float64)
    for c in range(NCORES):
        out[c // CPB] += res.results[c]["out"].astype(np.float64)
    return out.astype(np.float32)


def make_runner(x, mask, w_qkv, w_out):
    """Persistent jitted runner over 8 cores with device-resident inputs,
    for steady-state timing (mirrors bass2jax.run_bass_via_pjrt without
    output donation — this kernel writes every output element)."""
    import jax
    from jax.sharding import Mesh, PartitionSpec, NamedSharding
    from jax.experimental.shard_map import shard_map
    from concourse import bass2jax
    import concourse.mybir as mybir

    bass2jax.install_neuronx_cc_hook()
    status, is_causal, in_maps = _host_prep(x, mask, w_qkv, w_out)
    key = (is_causal, status.tobytes())
    if key not in _built:
        _built[key] = _build(status, is_causal)
    nc = _built[key]

    partition_name = (nc.partition_id_tensor.name
                      if nc.partition_id_tensor else None)
    in_names, out_names, out_avals = [], [], []
    for alloc in nc.m.functions[0].allocations:
        if not isinstance(alloc, mybir.MemoryLocationSet):
            continue
        name = alloc.memorylocations[0].name
        if alloc.kind == "ExternalInput":
            if name != partition_name:
                in_names.append(name)
        elif alloc.kind == "ExternalOutput":
            out_names.append(name)
            out_avals.append(jax.core.ShapedArray(
                tuple(alloc.tensor_shape), mybir.dt.np(alloc.dtype)))
    n_params = len(in_names)
    all_in_names = in_names + out_names
    if partition_name is not None:
        all_in_names.append(partition_name)

    def _body(*args):
        operands = list(args)
        if partition_name is not None:
            operands.append(bass2jax.partition_id_tensor())
        outs = bass2jax._bass_exec_p.bind(
            *operands, out_avals=tuple(out_avals), in_names=tuple(all_in_names),
            out_names=tuple(out_names), lowering_input_output_aliases=(),
            sim_require_finite=True, sim_require_nnan=True, nc=nc)
        return tuple(outs)

    devices = jax.devices()[:NCORES]
    mesh = Mesh(np.asarray(devices), ("core",))
    spec = NamedSharding(mesh, PartitionSpec("core"))
    sharded = jax.jit(
        shard_map(_body, mesh=mesh,
                  in_specs=(PartitionSpec("core"),) * (n_params + len(out_names)),
                  out_specs=(PartitionSpec("core"),) * len(out_names),
                  check_rep=False),
        keep_unused=True)
    concat_in = [
        jax.device_put(
            np.concatenate([in_maps[c][n] for c in range(NCORES)], 0), spec)
        for n in in_names]
    concat_zeros = [
        jax.device_put(
            np.zeros((NCORES * a.shape[0], *a.shape[1:]), a.dtype), spec)
        for a in out_avals]

    def run():
        return sharded(*concat_in, *concat_zeros)

    def collect(out_arrs):
        full = np.asarray(out_arrs[0]).reshape(NCORES, L, D)
        out = np.zeros((B, L, D), np.float64)
        for c in range(NCORES):
            out[c // CPB] += full[c]
        return out.astype(np.float32)

    return run, collect


# revision 14
# speedup vs baseline: 1.2840x; 1.2840x over previous
"""Multi-head causal attention (B=2, L=2048, D=1024, H=16, Hd=64) on 8 TRN2
NeuronCores.

Sharding: data-parallel over the 2 batches x tensor-parallel over heads
(4 cores per batch, 4 heads per core).  Each core computes its heads'
QKV projection, attention, and a partial out-projection over its 256
local dims; the host sums the 4 partials per batch.

All matmul operands are fp16 (full-rate PE streaming + FWL weight load,
half the HBM traffic); accumulation stays fp32 in PSUM.

Per-core dataflow (per head pair hp, bases 0/64 of the m-tiles):
  qT,kT  [512, L]  = wqkT.T @ xT           (scale 1/8 folded into wq)
  v      [L, 256]  = xT.T-tiles @ wvT      ([l,d] layout)
  S^T    [128k, 512q] = kT_h.T @ qT_h      (K=64, the pair's two heads on
         disjoint PE row groups run concurrently)
  E      = exp(S^T)                        (one ACT op per k-tile, both heads)
  av     [128, 512q] += v_h.T @ E_h        (M=64 col-tiled: the pair's two
         AV matmuls run concurrently on disjoint PE column groups)
  den    [4x1, 512q] += 1.T @ E            (4-way col-tiled M=1 matmuls --
         two k-tiles x two heads per pass -- every other k-tile)
  attnT  = av * reciprocal(den0 + den1)    (DVE fold + recip + gpsimd
         partition_broadcast + DVE mul straight out of PSUM)
  out    [L, 1024] += attnT-pair.T @ woT-pair    (K=128 per head pair)

Compared to the M=65 [v|1] ones-column trick, the col-tiled AV + packed
denominator pass cuts AV streaming cycles from 2x to 1.25x the S^T
cycles (the M=65 form wastes half the PE columns).

Scheduling: the ACT exp chain (~83us) and the PE stream (~87us) are both
near the kernel's critical path, so emission is paced by a greedy weave
that tracks estimated PE/ACT clocks: attention S^T steps are emitted as
late as the exp backlog allows, with projection / out-projection groups
(pure PE work) pulled from a requirement-tagged filler queue in between.
exp starts ~14us in (right after the first QKV group's dependencies
land) instead of after two full QKV chunks.  Input DMAs issue on the
sync/vector/gpsimd rings, critical tiles first; the scalar queue is kept
free for the exp stream.  One shared 8-bank PSUM pool: qkps 1 + vps 1 +
st 2x2 + av 1 + den 1 = 8; the out-projection reuses qkps/vps (and idle
st banks at the tail).
"""
import sys
sys.path.insert(0, '/opt/trn_rl_repo')
import numpy as np

B, L, D = 2, 2048, 1024
H, HD = 16, 64
NCORES = 8
CPB = 4              # cores per batch
HPC = H // CPB       # heads per core = 4
DLOC = HPC * HD      # 256 local head dims per core
NKT, NQT = L // 128, L // 512   # 16 k-tiles, 4 q-tiles
NEG = -30000.0

_built = {}


def _build(status, use_cb):
    """status: [NKT, NQT] int8 (0=skip, 1=full, 2=mixed); use_cb: causal
    on-chip bias patterns (True) vs DMA'd bias tiles (False)."""
    import concourse.mybir as mybir
    import concourse.tile as tile
    from concourse import bacc

    F32 = mybir.dt.float32
    F16 = mybir.dt.float16
    Exp = mybir.ActivationFunctionType.Exp

    # mixed-block index map for the DMA'd-bias mode
    mixed_ids = {}
    for qt in range(NQT):
        for kt in range(NKT):
            if status[kt, qt] == 2:
                mixed_ids[(kt, qt)] = len(mixed_ids)
    nmix = len(mixed_ids)

    nc = bacc.Bacc("TRN2", target_bir_lowering=False, debug=False)
    xT_d = nc.dram_tensor("xT", [D, L], F16, kind="ExternalInput")
    wqkT_d = nc.dram_tensor("wqkT", [D, 2 * DLOC], F16, kind="ExternalInput")
    wvT_d = nc.dram_tensor("wvT", [D, DLOC], F16, kind="ExternalInput")
    woT_d = nc.dram_tensor("woT", [128, 2 * D], F16, kind="ExternalInput")
    if not use_cb and nmix:
        bias_d = nc.dram_tensor("bias", [nmix, 128, 512], F32, kind="ExternalInput")
    out_d = nc.dram_tensor("out", [L, D], F16, kind="ExternalOutput")

    with tile.TileContext(nc) as tc:
        with tc.tile_pool(name="const", bufs=1) as const, \
             tc.tile_pool(name="esp", bufs=5) as esp, \
             tc.tile_pool(name="misc", bufs=2) as misc, \
             tc.tile_pool(name="otp", bufs=3) as otp:

            # ---- input loads: sync/vector/gpsimd issue rings (scalar is
            # reserved for the exp stream), critical tiles first: the first
            # attention unit needs wqk groups 0/2 + x quarter 0; wv right
            # behind for the v fillers, then the rest in use order ----
            wqr = wqkT_d.ap().rearrange("(a p) m -> p a m", p=128)
            wqkg = [const.tile([128, D // 128, 128], F16, tag=f"wqk{g}",
                               name=f"wqk{g}") for g in range(4)]
            xqt = [const.tile([128, D // 128, 512], F16, tag=f"xq{q}",
                              name=f"xq{q}") for q in range(4)]
            xr = xT_d.ap().rearrange("(a p) l -> p a l", p=128)
            wv = const.tile([128, D // 128, DLOC], F16, tag="wv")
            wo = const.tile([128, 2 * D], F16, tag="wo")

            # weights on the scalar ring (free until the first exp); each x
            # quarter split across the sync+gpsimd rings so the critical
            # quarter 0 moves at double bandwidth
            nc.scalar.dma_start(out=wqkg[0], in_=wqr[:, :, 0:128])
            nc.scalar.dma_start(out=wqkg[2], in_=wqr[:, :, 256:384])
            for q in range(4):
                c = slice(q * 512, (q + 1) * 512)
                nc.sync.dma_start(out=xqt[q][:, 0:4, :], in_=xr[:, 0:4, c])
                nc.gpsimd.dma_start(out=xqt[q][:, 4:8, :], in_=xr[:, 4:8, c])
                if q == 0:
                    nc.scalar.dma_start(
                        out=wv,
                        in_=wvT_d.ap().rearrange("(a p) m -> p a m", p=128))
                elif q == 1:
                    nc.scalar.dma_start(out=wqkg[1], in_=wqr[:, :, 128:256])
                    nc.scalar.dma_start(out=wqkg[3], in_=wqr[:, :, 384:512])
                elif q == 2:
                    nc.scalar.dma_start(out=wo, in_=woT_d.ap())

            def xslice(l0, l1):
                q = l0 // 512
                assert l1 <= (q + 1) * 512
                return lambda k: xqt[q][:, k, l0 - q * 512:l1 - q * 512]

            # ---- causal 0/1 mask patterns (r = kt - 4*qt in 0..3) ----
            if use_cb:
                cb = const.tile([128, 4, 512], F16, tag="cb")
                nc.vector.memset(cb, 1.0)
                for r in range(4):
                    # keep 1.0 where -k + q - 128r >= 0 (attend), else 0.0
                    nc.gpsimd.affine_select(
                        out=cb[:, r, :],
                        in_=cb[:, r, :],
                        compare_op=mybir.AluOpType.is_ge, fill=0.0,
                        base=-128 * r, channel_multiplier=-1,
                        pattern=[[1, 512]])

            # per-L-tile projection results; v carries a ones column per
            # head ([v_h | 1] stride 65) so the AV matmul accumulates the
            # softmax denominator with zero extra weight-load traffic
            qkl = [const.tile([128, 4, 512], F16, tag=f"qk{lt}",
                              name=f"qk{lt}")
                   for lt in range(NQT)]
            vtg = [const.tile([128, 4, HPC * (HD + 1)], F16, tag=f"vt{g}",
                              name=f"vt{g}")
                   for g in range(NQT)]
            for g in range(NQT):
                # fill with 1.0; the v copies below overwrite all but the
                # per-head ones-columns
                nc.vector.memset(vtg[g], 1.0)

            with tc.tile_pool(name="psum", bufs=1, space="PSUM") as psum, \
                 tc.tile_pool(name="atp", bufs=4) as atp:

                def qk_group(lt, g):
                    ps = psum.tile([128, 512], F32, tag="qkps", bufs=1,
                                   name=f"qkps{lt}{g}")
                    xs = xslice(lt * 512, (lt + 1) * 512)
                    for kt in range(D // 128):
                        nc.tensor.matmul(
                            ps, wqkg[g][:, kt, :], xs(kt),
                            start=(kt == 0), stop=(kt == D // 128 - 1))
                    nc.vector.tensor_copy(qkl[lt][:, g, :], ps)

                def v_group(lt, g):
                    l16 = 4 * lt + g
                    psv = psum.tile([128, DLOC], F32, tag="vps", bufs=1,
                                    name=f"vps{l16}")
                    xs = xslice(l16 * 128, (l16 + 1) * 128)
                    for kt in range(D // 128):
                        nc.tensor.matmul(
                            psv, xs(kt), wv[:, kt, :],
                            start=(kt == 0), stop=(kt == D // 128 - 1))
                    nc.vector.tensor_copy(
                        vtg[lt][:, g, :]
                        .rearrange("p (h c) -> p h c", c=HD + 1)[:, :, 0:HD],
                        psv.rearrange("p (h c) -> p h c", c=HD))

                # ---- attention unit (one q-tile x one head pair) ----
                # Steps: S(i) = S^T pair matmuls + exp; A(i) = the pair's
                # two [v_h|1] M=65 AV matmuls (denominator rides along as
                # row 64); norm = recip + bcast + mul.
                at_tiles = {}

                def unit_steps(qt, hp, extra_prenorm=None, last=False):
                    kts = [kt for kt in range(NKT) if status[kt, qt] != 0]
                    n = len(kts)
                    cell = {}
                    if qt not in at_tiles:
                        at_tiles[qt] = [
                            atp.tile([128, 512], F16, tag=f"at{p}",
                                     name=f"at{p}_{qt}") for p in range(2)]
                    mq, mk = hp, 2 + hp

                    def s_step(i):
                        kt = kts[i]
                        if i == 0:
                            cell["av"] = psum.tile(
                                [65, 2, 512], F32, tag="av", bufs=1,
                                name=f"av{qt}{hp}")
                        mixed = status[kt, qt] == 2
                        c0 = 128 * (kt - 4 * qt) if (mixed and use_cb) else 0
                        cell[("c0", i)] = c0
                        st = psum.tile([128, 2, 512], F32, tag="st",
                                       bufs=2, name=f"st{qt}{hp}{kt}")
                        for j, base in ((0, 0), (1, 64)):
                            nc.tensor.matmul(
                                st[:, j, c0:],
                                qkl[kt // 4][base:base + 64, mk,
                                             (kt % 4) * 128:
                                             (kt % 4 + 1) * 128],
                                qkl[qt][base:base + 64, mq, c0:],
                                start=True, stop=True)
                        if mixed and not use_cb:
                            b_ap = misc.tile([128, 512], F32, tag="bt")
                            nc.sync.dma_start(
                                out=b_ap,
                                in_=bias_d.ap()[mixed_ids[(kt, qt)]])
                            for j in range(2):
                                nc.vector.tensor_add(
                                    st[:, j, :], st[:, j, :], b_ap)
                        es = esp.tile([128, 2, 512], F16, tag="es")
                        cell[("es", i)] = es
                        nc.scalar.activation(es[:, :, c0:],
                                             st[:, :, c0:], Exp)
                        if mixed and use_cb:
                            # only the 128-wide diagonal strip is partial
                            nc.vector.tensor_mul(
                                es[:, :, c0:c0 + 128],
                                es[:, :, c0:c0 + 128],
                                cb[:, 0:1, 0:128].broadcast_to(
                                    [128, 2, 128]))

                    def a_step(i):
                        kt = kts[i]
                        c0 = cell[("c0", i)]
                        es = cell[("es", i)]
                        av = cell["av"]
                        for j, h in ((0, 2 * hp), (1, 2 * hp + 1)):
                            nc.tensor.matmul(
                                av[:, j, c0:],
                                vtg[kt // 4][:, kt % 4,
                                             h * (HD + 1):
                                             (h + 1) * (HD + 1)],
                                es[:, j, c0:],
                                start=(i == 0), stop=(i == n - 1),
                                skip_group_check=True)

                    def norm():
                        # attnT_h = av[0:64] / av[64].  custom-DVE ops and
                        # partition_broadcast read the physical tile start,
                        # so the denominator and its reciprocal live in
                        # base-0 tiles.
                        av = cell["av"]
                        if last:
                            # tail: the chain is fully exposed — pipeline
                            # the two head-pair halves across ACT (idle),
                            # DVE and gpsimd, multiplying out of PSUM
                            for j, base in ((0, 0), (1, 64)):
                                dcp = misc.tile([1, 512], F32,
                                                tag=f"dcpl{j}", bufs=1)
                                nc.scalar.copy(dcp, av[64:65, j, :])
                                rc = misc.tile([1, 512], F32,
                                               tag=f"rcl{j}", bufs=1)
                                nc.vector.reciprocal_approx_fast(
                                    out=rc, in_=dcp)
                                bc = misc.tile([64, 512], F32,
                                               tag=f"bcl{j}", bufs=1)
                                nc.gpsimd.partition_broadcast(
                                    bc, rc, channels=64)
                                nc.vector.tensor_mul(
                                    at_tiles[qt][hp][base:base + 64, :],
                                    av[0:64, j, :], bc)
                            return
                        dcp = misc.tile([1, 2, 512], F32, tag="dcp", bufs=2)
                        nc.vector.tensor_copy(dcp, av[64:65, :, :])
                        rc = misc.tile([1, 2, 512], F32, tag="rc", bufs=2)
                        nc.vector.reciprocal_approx_fast(out=rc, in_=dcp)
                        bc = misc.tile([64, 2, 512], F32, tag="bc", bufs=2)
                        nc.gpsimd.partition_broadcast(bc, rc, channels=64)
                        avs = misc.tile([64, 2, 512], F32, tag="avs",
                                        bufs=3)
                        nc.vector.tensor_copy(avs, av[0:64, :, :])
                        for j, base in ((0, 0), (1, 64)):
                            nc.vector.tensor_mul(
                                at_tiles[qt][hp][base:base + 64, :],
                                avs[0:64, j, :], bc[:, j, :])

                    # step list: (emit_fn, pe_ns, act_ns, requirements)
                    # A(i) trails S(i+1) so the exp pipeline stays ahead
                    steps = []

                    def est_w(i):
                        kt = kts[i]
                        mixed = status[kt, qt] == 2
                        return 512 - (128 * (kt - 4 * qt)
                                      if (mixed and use_cb) else 0)

                    for i in range(n):
                        kt = kts[i]
                        wd = est_w(i)
                        steps.append((
                            (lambda i=i: s_step(i)),
                            wd / 2.4,
                            (2 * wd + 352) / 1.2,
                            [("qk", qt, mq), ("qk", kt // 4, mk)]))
                        if i >= 1:
                            kp = kts[i - 1]
                            steps.append((
                                (lambda i=i - 1: a_step(i)),
                                2 * est_w(i - 1) / 2.4, 0.0,
                                [("v", kp // 4, kp % 4)]))
                    kp = kts[n - 1]
                    steps.append(((lambda: a_step(n - 1)),
                                  2 * est_w(n - 1) / 2.4, 0.0,
                                  [("v", kp // 4, kp % 4)]))
                    if extra_prenorm:
                        steps.append(extra_prenorm)
                    steps.append((norm, 0.0, 0.0, []))
                    return steps

                # ---- out-projection ----
                op3_pos = {}

                def op_lt_pass0(qt, lt, use_st=False):
                    if use_st:
                        t = psum.tile([128, 2, 512], F32, tag="st", bufs=2,
                                      name=f"post{qt}{lt}")
                        pos = [t[:, 0, :], t[:, 1, :]]
                    else:
                        pos = [psum.tile([128, 512], F32, tag=t, bufs=1,
                                         name=f"po{qt}{lt}{t}")
                               for t in ("qkps", "vps")]
                    op3_pos[(qt, lt)] = pos
                    for do in range(2):
                        nc.tensor.matmul(
                            pos[do],
                            at_tiles[qt][0][:, lt * 128:(lt + 1) * 128],
                            wo[:, do * 512:do * 512 + 512],
                            start=True, stop=False)

                def op_lt_pass1(qt, lt, tail=False):
                    pos = op3_pos.pop((qt, lt))
                    row = qt * 512 + lt * 128
                    ot = otp.tile([128, 2, 512], F16, tag="ot")
                    for do in range(2):
                        nc.tensor.matmul(
                            pos[do],
                            at_tiles[qt][1][:, lt * 128:(lt + 1) * 128],
                            wo[:, D + do * 512:D + do * 512 + 512],
                            start=False, stop=True)
                        if tail and do == 1:
                            # tail: ACT is idle; split the evacuation
                            # copies across ACT and DVE
                            nc.scalar.copy(ot[:, do, :], pos[do])
                        else:
                            nc.vector.tensor_copy(ot[:, do, :], pos[do])
                    if tail:
                        eng = nc.scalar if lt % 2 == 1 else nc.sync
                    else:
                        eng = nc.gpsimd if lt % 2 == 1 else nc.sync
                    eng.dma_start(
                        out=out_d.ap()[row:row + 128, :],
                        in_=ot.rearrange("p a b -> p (a b)"))

                def op_lt_step(qt, lt, tail=False):
                    op_lt_pass0(qt, lt, use_st=tail and lt % 2 == 1)
                    op_lt_pass1(qt, lt, tail)

                if use_cb:
                    # ---- greedy paced emission ----
                    # filler queue: (name, fn, pe_ns, req, min_ns)
                    fq = []
                    for lt in range(NQT):
                        arr = 11000.0 + 3500.0 * lt
                        for g in (0, 2, 1, 3):
                            fq.append((("qk", lt, g),
                                       (lambda lt=lt, g=g: qk_group(lt, g)),
                                       1707.0, None, arr))
                        for g in range(4):
                            fq.append((("v", lt, g),
                                       (lambda lt=lt, g=g: v_group(lt, g)),
                                       853.0, None, max(arr, 13000.0)))
                    for qt in range(NQT - 1):
                        for lt in range(4):
                            fq.append((("op", qt, lt),
                                       (lambda qt=qt, lt=lt:
                                        op_lt_step(qt, lt)),
                                       1707.0, ("unit", qt, 1), 22000.0))

                    emitted = set()
                    done_units = set()
                    clk = {"pe": 8300.0, "act": 13500.0}

                    def emit_filler(entry):
                        name, fn, pe_ns, req, min_ns = entry
                        fn()
                        emitted.add(name)
                        clk["pe"] += pe_ns
                        fq.remove(entry)

                    def ensure(name):
                        if name in emitted or name[0] == "unit":
                            return
                        for entry in fq:
                            if entry[0] == name:
                                emit_filler(entry)
                                return

                    def pull_filler():
                        # first ready filler in queue order
                        for entry in fq:
                            req, min_ns = entry[3], entry[4]
                            if req is not None and req not in done_units:
                                continue
                            if min_ns > clk["pe"] + 1500.0:
                                continue
                            return entry
                        return None

                    units = [(qt, hp) for qt in range(NQT) for hp in (0, 1)]
                    for qt, hp in units:
                        extra = None
                        if (qt, hp) == (3, 1):
                            # out-proj(3) pair-0 pass sits directly before
                            # the last normalize, covering its latency
                            extra = ((lambda: op_lt_pass0(3, 0,
                                                          use_st=True)),
                                     427.0, 0.0, [])
                        for fn, pe_ns, act_ns, reqs in unit_steps(
                                qt, hp, extra_prenorm=extra,
                                last=(qt, hp) == (3, 1)):
                            for r in reqs:
                                ensure(r)
                            if act_ns > 0.0:
                                # S-step: pad with fillers while the exp
                                # backlog can hide them
                                while True:
                                    gap = clk["act"] - (clk["pe"] + pe_ns)
                                    if gap <= 0.0:
                                        break
                                    entry = pull_filler()
                                    if entry is None or entry[2] > gap + 900.0:
                                        break
                                    emit_filler(entry)
                            fn()
                            clk["pe"] += pe_ns
                            if act_ns > 0.0:
                                clk["act"] = max(clk["act"],
                                                 clk["pe"]) + act_ns
                        done_units.add(("unit", qt, hp))

                    # flush leftovers (op(2) usually lands here)
                    while fq:
                        emit_filler(fq[0])
                    op_lt_pass1(3, 0, tail=True)
                    for lt in range(1, 4):
                        op_lt_step(3, lt, tail=True)
                else:
                    for lt in range(NQT):
                        for g in (0, 2, 1, 3):
                            qk_group(lt, g)
                        for g in range(4):
                            v_group(lt, g)
                    for qt in range(NQT):
                        for hp in (0, 1):
                            for fn, _, _, _ in unit_steps(
                                    qt, hp,
                                    last=(qt, hp) == (NQT - 1, 1)):
                                fn()
                        for lt in range(4):
                            op_lt_step(qt, lt, tail=(qt == NQT - 1))
    nc.compile()
    return nc


def _host_prep(x, mask, w_qkv, w_out):
    x = np.asarray(x, dtype=np.float32)
    mask = np.asarray(mask).astype(bool)
    w_qkv = np.asarray(w_qkv, dtype=np.float32)
    w_out = np.asarray(w_out, dtype=np.float32)

    tril = np.tril(np.ones((L, L), dtype=bool))
    is_causal = all(np.array_equal(mask[b], tril) for b in range(B))

    # block classification on the S^T layout: block (kt, qt) covers
    # k in [kt*128, ...), q in [qt*512, ...)
    status = np.zeros((NKT, NQT), np.int8)
    if is_causal:
        for qt in range(NQT):
            for kt in range(NKT):
                r = kt - 4 * qt
                status[kt, qt] = 0 if r > 3 else (2 if r >= 0 else 1)
    else:
        for qt in range(NQT):
            for kt in range(NKT):
                blk = mask[:, qt * 512:(qt + 1) * 512, kt * 128:(kt + 1) * 128]
                status[kt, qt] = 1 if blk.all() else (0 if not blk.any() else 2)

    # per-core inputs
    scale = float(HD) ** -0.5
    in_maps = []
    bias_by_batch = None
    if not is_causal:
        mixed = [(kt, qt) for qt in range(NQT) for kt in range(NKT)
                 if status[kt, qt] == 2]
        if mixed:
            bias_by_batch = []
            for b in range(B):
                tiles = np.zeros((len(mixed), 128, 512), np.float32)
                mt = mask[b].T  # [k, q]
                for i, (kt, qt) in enumerate(mixed):
                    blk = mt[kt * 128:(kt + 1) * 128, qt * 512:(qt + 1) * 512]
                    tiles[i] = np.where(blk, 0.0, NEG)
                bias_by_batch.append(tiles)

    for c in range(NCORES):
        b = c // CPB
        hq = (c % CPB) * HPC
        wq = w_qkv[hq * HD:(hq + HPC) * HD] * scale
        wk = w_qkv[D + hq * HD:D + (hq + HPC) * HD]
        wv = w_qkv[2 * D + hq * HD:2 * D + (hq + HPC) * HD]
        wqkT = np.ascontiguousarray(
            np.concatenate([wq, wk], 0).T.astype(np.float16))
        wvT = np.ascontiguousarray(wv.T.astype(np.float16))
        wo_loc = w_out[:, hq * HD:(hq + HPC) * HD].T       # [256, 1024]
        woT = np.ascontiguousarray(
            wo_loc.reshape(2, 128, D).transpose(1, 0, 2)
            .reshape(128, 2 * D).astype(np.float16))
        im = {"xT": np.ascontiguousarray(x[b].T.astype(np.float16)),
              "wqkT": wqkT, "wvT": wvT, "woT": woT}
        if bias_by_batch is not None:
            im["bias"] = bias_by_batch[b]
        in_maps.append(im)
    return status, is_causal, in_maps


LAST_RESULTS = None


def kernel(x, mask, w_qkv, w_out):
    from concourse.bass_utils import run_bass_kernel_spmd
    global LAST_RESULTS

    status, is_causal, in_maps = _host_prep(x, mask, w_qkv, w_out)
    key = (is_causal, status.tobytes())
    if key not in _built:
        _built[key] = _build(status, is_causal)
    nc = _built[key]

    res = run_bass_kernel_spmd(nc, in_maps, core_ids=list(range(NCORES)))
    LAST_RESULTS = res
    out = np.zeros((B, L, D), np.float64)
    for c in range(NCORES):
        out[c // CPB] += res.results[c]["out"].astype(np.float64)
    return out.astype(np.float32)


def make_runner(x, mask, w_qkv, w_out):
    """Persistent jitted runner over 8 cores with device-resident inputs,
    for steady-state timing (mirrors bass2jax.run_bass_via_pjrt without
    output donation — this kernel writes every output element)."""
    import jax
    from jax.sharding import Mesh, PartitionSpec, NamedSharding
    from jax.experimental.shard_map import shard_map
    from concourse import bass2jax
    import concourse.mybir as mybir

    bass2jax.install_neuronx_cc_hook()
    status, is_causal, in_maps = _host_prep(x, mask, w_qkv, w_out)
    key = (is_causal, status.tobytes())
    if key not in _built:
        _built[key] = _build(status, is_causal)
    nc = _built[key]

    partition_name = (nc.partition_id_tensor.name
                      if nc.partition_id_tensor else None)
    in_names, out_names, out_avals = [], [], []
    for alloc in nc.m.functions[0].allocations:
        if not isinstance(alloc, mybir.MemoryLocationSet):
            continue
        name = alloc.memorylocations[0].name
        if alloc.kind == "ExternalInput":
            if name != partition_name:
                in_names.append(name)
        elif alloc.kind == "ExternalOutput":
            out_names.append(name)
            out_avals.append(jax.core.ShapedArray(
                tuple(alloc.tensor_shape), mybir.dt.np(alloc.dtype)))
    n_params = len(in_names)
    all_in_names = in_names + out_names
    if partition_name is not None:
        all_in_names.append(partition_name)

    def _body(*args):
        operands = list(args)
        if partition_name is not None:
            operands.append(bass2jax.partition_id_tensor())
        outs = bass2jax._bass_exec_p.bind(
            *operands, out_avals=tuple(out_avals), in_names=tuple(all_in_names),
            out_names=tuple(out_names), lowering_input_output_aliases=(),
            sim_require_finite=True, sim_require_nnan=True, nc=nc)
        return tuple(outs)

    devices = jax.devices()[:NCORES]
    mesh = Mesh(np.asarray(devices), ("core",))
    spec = NamedSharding(mesh, PartitionSpec("core"))
    sharded = jax.jit(
        shard_map(_body, mesh=mesh,
                  in_specs=(PartitionSpec("core"),) * (n_params + len(out_names)),
                  out_specs=(PartitionSpec("core"),) * len(out_names),
                  check_rep=False),
        keep_unused=True)
    concat_in = [
        jax.device_put(
            np.concatenate([in_maps[c][n] for c in range(NCORES)], 0), spec)
        for n in in_names]
    concat_zeros = [
        jax.device_put(
            np.zeros((NCORES * a.shape[0], *a.shape[1:]), a.dtype), spec)
        for a in out_avals]

    def run():
        return sharded(*concat_in, *concat_zeros)

    def collect(out_arrs):
        full = np.asarray(out_arrs[0]).reshape(NCORES, L, D)
        out = np.zeros((B, L, D), np.float64)
        for c in range(NCORES):
            out[c // CPB] += full[c]
        return out.astype(np.float32)

    return run, collect


# revision 16
# speedup vs baseline: 1.3249x; 1.0318x over previous
"""Multi-head causal attention (B=2, L=2048, D=1024, H=16, Hd=64) on 8 TRN2
NeuronCores.

Sharding: data-parallel over the 2 batches x tensor-parallel over heads
(4 cores per batch, 4 heads per core).  Each core computes its heads'
QKV projection, attention, and a partial out-projection over its 256
local dims; the host sums the 4 partials per batch.

All matmul operands are fp16 (full-rate PE streaming + FWL weight load,
half the HBM traffic); accumulation stays fp32 in PSUM.

Per-core dataflow (per head pair hp, bases 0/64 of the m-tiles):
  qT,kT  [512, L]  = wqkT.T @ xT           (scale 1/8 folded into wq)
  v      [L, 256]  = xT.T-tiles @ wvT      ([l,d] layout)
  S^T    [128k, 512q] = kT_h.T @ qT_h      (K=64, the pair's two heads on
         disjoint PE row groups run concurrently)
  E      = exp(S^T)                        (one ACT op per k-tile, both heads)
  av     [128, 512q] += v_h.T @ E_h        (M=64 col-tiled: the pair's two
         AV matmuls run concurrently on disjoint PE column groups)
  den    [4x1, 512q] += 1.T @ E            (4-way col-tiled M=1 matmuls --
         two k-tiles x two heads per pass -- every other k-tile)
  attnT  = av * reciprocal(den0 + den1)    (DVE fold + recip + gpsimd
         partition_broadcast + DVE mul straight out of PSUM)
  out    [L, 1024] += attnT-pair.T @ woT-pair    (K=128 per head pair)

Compared to the M=65 [v|1] ones-column trick, the col-tiled AV + packed
denominator pass cuts AV streaming cycles from 2x to 1.25x the S^T
cycles (the M=65 form wastes half the PE columns).

Scheduling: the ACT exp chain (~83us) and the PE stream (~87us) are both
near the kernel's critical path, so emission is paced by a greedy weave
that tracks estimated PE/ACT clocks: attention S^T steps are emitted as
late as the exp backlog allows, with projection / out-projection groups
(pure PE work) pulled from a requirement-tagged filler queue in between.
exp starts ~14us in (right after the first QKV group's dependencies
land) instead of after two full QKV chunks.  Input DMAs issue on the
sync/vector/gpsimd rings, critical tiles first; the scalar queue is kept
free for the exp stream.  One shared 8-bank PSUM pool: qkps 1 + vps 1 +
st 2x2 + av 1 + den 1 = 8; the out-projection reuses qkps/vps (and idle
st banks at the tail).
"""
import sys
sys.path.insert(0, '/opt/trn_rl_repo')
import numpy as np

B, L, D = 2, 2048, 1024
H, HD = 16, 64
NCORES = 8
CPB = 4              # cores per batch
HPC = H // CPB       # heads per core = 4
DLOC = HPC * HD      # 256 local head dims per core
NKT, NQT = L // 128, L // 512   # 16 k-tiles, 4 q-tiles
NEG = -30000.0

_built = {}


def _build(status, use_cb):
    """status: [NKT, NQT] int8 (0=skip, 1=full, 2=mixed); use_cb: causal
    on-chip bias patterns (True) vs DMA'd bias tiles (False)."""
    import concourse.mybir as mybir
    import concourse.tile as tile
    from concourse import bacc

    F32 = mybir.dt.float32
    F16 = mybir.dt.float16
    Exp = mybir.ActivationFunctionType.Exp

    # mixed-block index map for the DMA'd-bias mode
    mixed_ids = {}
    for qt in range(NQT):
        for kt in range(NKT):
            if status[kt, qt] == 2:
                mixed_ids[(kt, qt)] = len(mixed_ids)
    nmix = len(mixed_ids)

    nc = bacc.Bacc("TRN2", target_bir_lowering=False, debug=False)
    xT_d = nc.dram_tensor("xT", [D, L], F16, kind="ExternalInput")
    wqkT_d = nc.dram_tensor("wqkT", [D, 2 * DLOC], F16, kind="ExternalInput")
    wvT_d = nc.dram_tensor("wvT", [D, DLOC], F16, kind="ExternalInput")
    woT_d = nc.dram_tensor("woT", [128, 2 * D], F16, kind="ExternalInput")
    if not use_cb and nmix:
        bias_d = nc.dram_tensor("bias", [nmix, 128, 512], F32, kind="ExternalInput")
    out_d = nc.dram_tensor("out", [L, D], F16, kind="ExternalOutput")

    with tile.TileContext(nc) as tc:
        with tc.tile_pool(name="const", bufs=1) as const, \
             tc.tile_pool(name="esp", bufs=5) as esp, \
             tc.tile_pool(name="misc", bufs=2) as misc, \
             tc.tile_pool(name="otp", bufs=3) as otp:

            # ---- input loads: sync/vector/gpsimd issue rings (scalar is
            # reserved for the exp stream), critical tiles first: the first
            # attention unit needs wqk groups 0/2 + x quarter 0; wv right
            # behind for the v fillers, then the rest in use order ----
            # total early DMA bandwidth is fabric-capped (~230 GB/s), so
            # strict critical-first ordering matters more than ring count:
            # wqk groups 0/2 + x quarter 0 + wv feed the first attention
            # unit.  Rings: sync + gpsimd (the scalar queue stays free for
            # the exp stream).
            wqr = wqkT_d.ap().rearrange("(a p) m -> p a m", p=128)
            wqkg = [const.tile([128, D // 128, 128], F16, tag=f"wqk{g}",
                               name=f"wqk{g}") for g in range(4)]
            xq = [[const.tile([128, 512], F16, tag=f"xq{k}_{q}",
                              name=f"xq{k}_{q}")
                   for q in range(4)] for k in range(D // 128)]
            xr = xT_d.ap().rearrange("(a p) l -> a p l", p=128)
            wv = const.tile([128, D // 128, DLOC], F16, tag="wv")
            wo = const.tile([128, 2 * D], F16, tag="wo")
            nc.gpsimd.dma_start(out=wqkg[0], in_=wqr[:, :, 0:128])
            for q in range(4):
                for k in range(D // 128):
                    eng = nc.sync if k % 2 == 0 else nc.gpsimd
                    eng.dma_start(out=xq[k][q],
                                  in_=xr[k][:, q * 512:(q + 1) * 512])
                    if q == 0 and k == 1:
                        nc.gpsimd.dma_start(out=wqkg[2],
                                            in_=wqr[:, :, 256:384])
                if q == 0:
                    nc.sync.dma_start(
                        out=wv,
                        in_=wvT_d.ap().rearrange("(a p) m -> p a m", p=128))
                    nc.gpsimd.dma_start(out=wqkg[1], in_=wqr[:, :, 128:256])
                    nc.sync.dma_start(out=wqkg[3], in_=wqr[:, :, 384:512])
                if q == 1:
                    nc.gpsimd.dma_start(out=wo, in_=woT_d.ap())

            def xslice(l0, l1):
                q = l0 // 512
                assert l1 <= (q + 1) * 512
                return lambda k: xq[k][q][:, l0 - q * 512:l1 - q * 512]

            # ---- causal 0/1 mask patterns (r = kt - 4*qt in 0..3) ----
            if use_cb:
                cb = const.tile([128, 4, 512], F16, tag="cb")
                nc.vector.memset(cb, 1.0)
                for r in range(4):
                    # keep 1.0 where -k + q - 128r >= 0 (attend), else 0.0
                    nc.gpsimd.affine_select(
                        out=cb[:, r, :],
                        in_=cb[:, r, :],
                        compare_op=mybir.AluOpType.is_ge, fill=0.0,
                        base=-128 * r, channel_multiplier=-1,
                        pattern=[[1, 512]])

            # per-L-tile projection results; v carries a ones column per
            # head ([v_h | 1] stride 65) so the AV matmul accumulates the
            # softmax denominator with zero extra weight-load traffic
            qkl = [const.tile([128, 4, 512], F16, tag=f"qk{lt}",
                              name=f"qk{lt}")
                   for lt in range(NQT)]
            vtg = [const.tile([128, 4, HPC * (HD + 1)], F16, tag=f"vt{g}",
                              name=f"vt{g}")
                   for g in range(NQT)]
            for g in range(NQT):
                # fill with 1.0; the v copies below overwrite all but the
                # per-head ones-columns
                nc.vector.memset(vtg[g], 1.0)

            with tc.tile_pool(name="psum", bufs=1, space="PSUM") as psum, \
                 tc.tile_pool(name="atp", bufs=4) as atp:

                def qk_group(lt, g):
                    ps = psum.tile([128, 512], F32, tag="qkps", bufs=1,
                                   name=f"qkps{lt}{g}")
                    xs = xslice(lt * 512, (lt + 1) * 512)
                    for kt in range(D // 128):
                        nc.tensor.matmul(
                            ps, wqkg[g][:, kt, :], xs(kt),
                            start=(kt == 0), stop=(kt == D // 128 - 1))
                    nc.vector.tensor_copy(qkl[lt][:, g, :], ps)

                def v_group(lt, g):
                    l16 = 4 * lt + g
                    psv = psum.tile([128, DLOC], F32, tag="vps", bufs=1,
                                    name=f"vps{l16}")
                    xs = xslice(l16 * 128, (l16 + 1) * 128)
                    for kt in range(D // 128):
                        nc.tensor.matmul(
                            psv, xs(kt), wv[:, kt, :],
                            start=(kt == 0), stop=(kt == D // 128 - 1))
                    nc.vector.tensor_copy(
                        vtg[lt][:, g, :]
                        .rearrange("p (h c) -> p h c", c=HD + 1)[:, :, 0:HD],
                        psv.rearrange("p (h c) -> p h c", c=HD))

                # ---- attention unit (one q-tile x one head pair) ----
                # Steps: S(i) = S^T pair matmuls + exp; A(i) = the pair's
                # two [v_h|1] M=65 AV matmuls (denominator rides along as
                # row 64); norm = recip + bcast + mul.
                at_tiles = {}

                def unit_steps(qt, hp, extra_prenorm=None, last=False):
                    kts = [kt for kt in range(NKT) if status[kt, qt] != 0]
                    n = len(kts)
                    cell = {}
                    if qt not in at_tiles:
                        at_tiles[qt] = [
                            atp.tile([128, 512], F16, tag=f"at{p}",
                                     name=f"at{p}_{qt}") for p in range(2)]
                    mq, mk = hp, 2 + hp

                    def s_step(i):
                        kt = kts[i]
                        if i == 0:
                            cell["av"] = psum.tile(
                                [65, 2, 512], F32, tag="av", bufs=1,
                                name=f"av{qt}{hp}")
                        mixed = status[kt, qt] == 2
                        c0 = 128 * (kt - 4 * qt) if (mixed and use_cb) else 0
                        cell[("c0", i)] = c0
                        st = psum.tile([128, 2, 512], F32, tag="st",
                                       bufs=2, name=f"st{qt}{hp}{kt}")
                        for j, base in ((0, 0), (1, 64)):
                            nc.tensor.matmul(
                                st[:, j, c0:],
                                qkl[kt // 4][base:base + 64, mk,
                                             (kt % 4) * 128:
                                             (kt % 4 + 1) * 128],
                                qkl[qt][base:base + 64, mq, c0:],
                                start=True, stop=True)
                        if mixed and not use_cb:
                            b_ap = misc.tile([128, 512], F32, tag="bt")
                            nc.sync.dma_start(
                                out=b_ap,
                                in_=bias_d.ap()[mixed_ids[(kt, qt)]])
                            for j in range(2):
                                nc.vector.tensor_add(
                                    st[:, j, :], st[:, j, :], b_ap)
                        es = esp.tile([128, 2, 512], F16, tag="es")
                        cell[("es", i)] = es
                        nc.scalar.activation(es[:, :, c0:],
                                             st[:, :, c0:], Exp)
                        if mixed and use_cb:
                            # only the 128-wide diagonal strip is partial
                            nc.vector.tensor_mul(
                                es[:, :, c0:c0 + 128],
                                es[:, :, c0:c0 + 128],
                                cb[:, 0:1, 0:128].broadcast_to(
                                    [128, 2, 128]))

                    def a_step(i):
                        kt = kts[i]
                        c0 = cell[("c0", i)]
                        es = cell[("es", i)]
                        av = cell["av"]
                        for j, h in ((0, 2 * hp), (1, 2 * hp + 1)):
                            nc.tensor.matmul(
                                av[:, j, c0:],
                                vtg[kt // 4][:, kt % 4,
                                             h * (HD + 1):
                                             (h + 1) * (HD + 1)],
                                es[:, j, c0:],
                                start=(i == 0), stop=(i == n - 1),
                                skip_group_check=True)

                    def norm():
                        # attnT_h = av[0:64] / av[64].  custom-DVE ops and
                        # partition_broadcast read the physical tile start,
                        # so the denominator and its reciprocal live in
                        # base-0 tiles.
                        av = cell["av"]
                        if last:
                            # tail: the chain is fully exposed — pipeline
                            # the two head-pair halves across ACT (idle),
                            # DVE and gpsimd, multiplying out of PSUM
                            for j, base in ((0, 0), (1, 64)):
                                dcp = misc.tile([1, 512], F32,
                                                tag=f"dcpl{j}", bufs=1)
                                nc.scalar.copy(dcp, av[64:65, j, :])
                                rc = misc.tile([1, 512], F32,
                                               tag=f"rcl{j}", bufs=1)
                                nc.vector.reciprocal_approx_fast(
                                    out=rc, in_=dcp)
                                bc = misc.tile([64, 512], F32,
                                               tag=f"bcl{j}", bufs=1)
                                nc.gpsimd.partition_broadcast(
                                    bc, rc, channels=64)
                                nc.vector.tensor_mul(
                                    at_tiles[qt][hp][base:base + 64, :],
                                    av[0:64, j, :], bc)
                            return
                        dcp = misc.tile([1, 2, 512], F32, tag="dcp", bufs=2)
                        nc.vector.tensor_copy(dcp, av[64:65, :, :])
                        rc = misc.tile([1, 2, 512], F32, tag="rc", bufs=2)
                        nc.vector.reciprocal_approx_fast(out=rc, in_=dcp)
                        bc = misc.tile([64, 2, 512], F32, tag="bc", bufs=2)
                        nc.gpsimd.partition_broadcast(bc, rc, channels=64)
                        avs = misc.tile([64, 2, 512], F32, tag="avs",
                                        bufs=3)
                        nc.vector.tensor_copy(avs, av[0:64, :, :])
                        for j, base in ((0, 0), (1, 64)):
                            nc.vector.tensor_mul(
                                at_tiles[qt][hp][base:base + 64, :],
                                avs[0:64, j, :], bc[:, j, :])

                    # step list: (emit_fn, pe_ns, act_ns, requirements)
                    # A(i) trails S(i+1) so the exp pipeline stays ahead
                    steps = []

                    def est_w(i):
                        kt = kts[i]
                        mixed = status[kt, qt] == 2
                        return 512 - (128 * (kt - 4 * qt)
                                      if (mixed and use_cb) else 0)

                    for i in range(n):
                        kt = kts[i]
                        wd = est_w(i)
                        steps.append((
                            (lambda i=i: s_step(i)),
                            wd / 2.4,
                            (2 * wd + 352) / 1.2,
                            [("qk", qt, mq), ("qk", kt // 4, mk)]))
                        if i >= 1:
                            kp = kts[i - 1]
                            steps.append((
                                (lambda i=i - 1: a_step(i)),
                                2 * est_w(i - 1) / 2.4, 0.0,
                                [("v", kp // 4, kp % 4)]))
                    kp = kts[n - 1]
                    steps.append(((lambda: a_step(n - 1)),
                                  2 * est_w(n - 1) / 2.4, 0.0,
                                  [("v", kp // 4, kp % 4)]))
                    if extra_prenorm:
                        steps.append(extra_prenorm)
                    steps.append((norm, 0.0, 0.0, []))
                    return steps

                # ---- out-projection ----
                op3_pos = {}

                def op_lt_pass0(qt, lt, use_st=False):
                    if use_st:
                        t = psum.tile([128, 2, 512], F32, tag="st", bufs=2,
                                      name=f"post{qt}{lt}")
                        pos = [t[:, 0, :], t[:, 1, :]]
                    else:
                        pos = [psum.tile([128, 512], F32, tag=t, bufs=1,
                                         name=f"po{qt}{lt}{t}")
                               for t in ("qkps", "vps")]
                    op3_pos[(qt, lt)] = pos
                    for do in range(2):
                        nc.tensor.matmul(
                            pos[do],
                            at_tiles[qt][0][:, lt * 128:(lt + 1) * 128],
                            wo[:, do * 512:do * 512 + 512],
                            start=True, stop=False)

                def op_lt_pass1(qt, lt, tail=False):
                    pos = op3_pos.pop((qt, lt))
                    row = qt * 512 + lt * 128
                    ot = otp.tile([128, 2, 512], F16, tag="ot")
                    for do in range(2):
                        nc.tensor.matmul(
                            pos[do],
                            at_tiles[qt][1][:, lt * 128:(lt + 1) * 128],
                            wo[:, D + do * 512:D + do * 512 + 512],
                            start=False, stop=True)
                        if tail and do == 1:
                            # tail: ACT is idle; split the evacuation
                            # copies across ACT and DVE
                            nc.scalar.copy(ot[:, do, :], pos[do])
                        else:
                            nc.vector.tensor_copy(ot[:, do, :], pos[do])
                    if tail:
                        eng = nc.scalar if lt % 2 == 1 else nc.sync
                    else:
                        eng = nc.gpsimd if lt % 2 == 1 else nc.sync
                    eng.dma_start(
                        out=out_d.ap()[row:row + 128, :],
                        in_=ot.rearrange("p a b -> p (a b)"))

                def op_lt_step(qt, lt, tail=False):
                    op_lt_pass0(qt, lt, use_st=tail and lt % 2 == 1)
                    op_lt_pass1(qt, lt, tail)

                if use_cb:
                    # ---- greedy paced emission ----
                    # filler queue: (name, fn, pe_ns, req, min_ns)
                    fq = []
                    for lt in range(NQT):
                        arr = 12000.0 + 6000.0 * lt
                        for g in (0, 2, 1, 3):
                            fq.append((("qk", lt, g),
                                       (lambda lt=lt, g=g: qk_group(lt, g)),
                                       1707.0, None, arr))
                        for g in range(4):
                            fq.append((("v", lt, g),
                                       (lambda lt=lt, g=g: v_group(lt, g)),
                                       853.0, None, max(arr, 15500.0)))
                    for qt in range(NQT - 1):
                        for lt in range(4):
                            fq.append((("op", qt, lt),
                                       (lambda qt=qt, lt=lt:
                                        op_lt_step(qt, lt)),
                                       1707.0, ("unit", qt, 1), 30000.0))

                    emitted = set()
                    done_units = set()
                    clk = {"pe": 8300.0, "act": 17000.0}

                    def emit_filler(entry):
                        name, fn, pe_ns, req, min_ns = entry
                        fn()
                        emitted.add(name)
                        clk["pe"] += pe_ns
                        fq.remove(entry)

                    def ensure(name):
                        if name in emitted or name[0] == "unit":
                            return
                        for entry in fq:
                            if entry[0] == name:
                                emit_filler(entry)
                                return

                    def pull_filler():
                        # first ready filler in queue order
                        for entry in fq:
                            req, min_ns = entry[3], entry[4]
                            if req is not None and req not in done_units:
                                continue
                            if min_ns > clk["pe"] + 1500.0:
                                continue
                            return entry
                        return None

                    units = [(qt, hp) for qt in range(NQT) for hp in (0, 1)]
                    for qt, hp in units:
                        extra = None
                        if (qt, hp) == (3, 1):
                            # out-proj(3) pair-0 pass sits directly before
                            # the last normalize, covering its latency
                            extra = ((lambda: op_lt_pass0(3, 0,
                                                          use_st=True)),
                                     427.0, 0.0, [])
                        for fn, pe_ns, act_ns, reqs in unit_steps(
                                qt, hp, extra_prenorm=extra,
                                last=(qt, hp) == (3, 1)):
                            for r in reqs:
                                ensure(r)
                            if act_ns > 0.0:
                                # S-step: pad with fillers while the exp
                                # backlog can hide them
                                while True:
                                    gap = clk["act"] - (clk["pe"] + pe_ns)
                                    if gap <= 0.0:
                                        break
                                    entry = pull_filler()
                                    if entry is None or entry[2] > gap + 900.0:
                                        break
                                    emit_filler(entry)
                            fn()
                            clk["pe"] += pe_ns
                            if act_ns > 0.0:
                                clk["act"] = max(clk["act"],
                                                 clk["pe"]) + act_ns
                        done_units.add(("unit", qt, hp))

                    # flush leftovers (op(2) usually lands here)
                    while fq:
                        emit_filler(fq[0])
                    op_lt_pass1(3, 0, tail=True)
                    for lt in range(1, 4):
                        op_lt_step(3, lt, tail=True)
                else:
                    for lt in range(NQT):
                        for g in (0, 2, 1, 3):
                            qk_group(lt, g)
                        for g in range(4):
                            v_group(lt, g)
                    for qt in range(NQT):
                        for hp in (0, 1):
                            for fn, _, _, _ in unit_steps(
                                    qt, hp,
                                    last=(qt, hp) == (NQT - 1, 1)):
                                fn()
                        for lt in range(4):
                            op_lt_step(qt, lt, tail=(qt == NQT - 1))
    nc.compile()
    return nc


def _host_prep(x, mask, w_qkv, w_out):
    x = np.asarray(x, dtype=np.float32)
    mask = np.asarray(mask).astype(bool)
    w_qkv = np.asarray(w_qkv, dtype=np.float32)
    w_out = np.asarray(w_out, dtype=np.float32)

    tril = np.tril(np.ones((L, L), dtype=bool))
    is_causal = all(np.array_equal(mask[b], tril) for b in range(B))

    # block classification on the S^T layout: block (kt, qt) covers
    # k in [kt*128, ...), q in [qt*512, ...)
    status = np.zeros((NKT, NQT), np.int8)
    if is_causal:
        for qt in range(NQT):
            for kt in range(NKT):
                r = kt - 4 * qt
                status[kt, qt] = 0 if r > 3 else (2 if r >= 0 else 1)
    else:
        for qt in range(NQT):
            for kt in range(NKT):
                blk = mask[:, qt * 512:(qt + 1) * 512, kt * 128:(kt + 1) * 128]
                status[kt, qt] = 1 if blk.all() else (0 if not blk.any() else 2)

    # per-core inputs
    scale = float(HD) ** -0.5
    in_maps = []
    bias_by_batch = None
    if not is_causal:
        mixed = [(kt, qt) for qt in range(NQT) for kt in range(NKT)
                 if status[kt, qt] == 2]
        if mixed:
            bias_by_batch = []
            for b in range(B):
                tiles = np.zeros((len(mixed), 128, 512), np.float32)
                mt = mask[b].T  # [k, q]
                for i, (kt, qt) in enumerate(mixed):
                    blk = mt[kt * 128:(kt + 1) * 128, qt * 512:(qt + 1) * 512]
                    tiles[i] = np.where(blk, 0.0, NEG)
                bias_by_batch.append(tiles)

    for c in range(NCORES):
        b = c // CPB
        hq = (c % CPB) * HPC
        wq = w_qkv[hq * HD:(hq + HPC) * HD] * scale
        wk = w_qkv[D + hq * HD:D + (hq + HPC) * HD]
        wv = w_qkv[2 * D + hq * HD:2 * D + (hq + HPC) * HD]
        wqkT = np.ascontiguousarray(
            np.concatenate([wq, wk], 0).T.astype(np.float16))
        wvT = np.ascontiguousarray(wv.T.astype(np.float16))
        wo_loc = w_out[:, hq * HD:(hq + HPC) * HD].T       # [256, 1024]
        woT = np.ascontiguousarray(
            wo_loc.reshape(2, 128, D).transpose(1, 0, 2)
            .reshape(128, 2 * D).astype(np.float16))
        im = {"xT": np.ascontiguousarray(x[b].T.astype(np.float16)),
              "wqkT": wqkT, "wvT": wvT, "woT": woT}
        if bias_by_batch is not None:
            im["bias"] = bias_by_batch[b]
        in_maps.append(im)
    return status, is_causal, in_maps


LAST_RESULTS = None


def kernel(x, mask, w_qkv, w_out):
    from concourse.bass_utils import run_bass_kernel_spmd
    global LAST_RESULTS

    status, is_causal, in_maps = _host_prep(x, mask, w_qkv, w_out)
    key = (is_causal, status.tobytes())
    if key not in _built:
        _built[key] = _build(status, is_causal)
    nc = _built[key]

    res = run_bass_kernel_spmd(nc, in_maps, core_ids=list(range(NCORES)))
    LAST_RESULTS = res
    out = np.zeros((B, L, D), np.float64)
    for c in range(NCORES):
        out[c // CPB] += res.results[c]["out"].astype(np.float64)
    return out.astype(np.float32)


def make_runner(x, mask, w_qkv, w_out):
    """Persistent jitted runner over 8 cores with device-resident inputs,
    for steady-state timing (mirrors bass2jax.run_bass_via_pjrt without
    output donation — this kernel writes every output element)."""
    import jax
    from jax.sharding import Mesh, PartitionSpec, NamedSharding
    from jax.experimental.shard_map import shard_map
    from concourse import bass2jax
    import concourse.mybir as mybir

    bass2jax.install_neuronx_cc_hook()
    status, is_causal, in_maps = _host_prep(x, mask, w_qkv, w_out)
    key = (is_causal, status.tobytes())
    if key not in _built:
        _built[key] = _build(status, is_causal)
    nc = _built[key]

    partition_name = (nc.partition_id_tensor.name
                      if nc.partition_id_tensor else None)
    in_names, out_names, out_avals = [], [], []
    for alloc in nc.m.functions[0].allocations:
        if not isinstance(alloc, mybir.MemoryLocationSet):
            continue
        name = alloc.memorylocations[0].name
        if alloc.kind == "ExternalInput":
            if name != partition_name:
                in_names.append(name)
        elif alloc.kind == "ExternalOutput":
            out_names.append(name)
            out_avals.append(jax.core.ShapedArray(
                tuple(alloc.tensor_shape), mybir.dt.np(alloc.dtype)))
    n_params = len(in_names)
    all_in_names = in_names + out_names
    if partition_name is not None:
        all_in_names.append(partition_name)

    def _body(*args):
        operands = list(args)
        if partition_name is not None:
            operands.append(bass2jax.partition_id_tensor())
        outs = bass2jax._bass_exec_p.bind(
            *operands, out_avals=tuple(out_avals), in_names=tuple(all_in_names),
            out_names=tuple(out_names), lowering_input_output_aliases=(),
            sim_require_finite=True, sim_require_nnan=True, nc=nc)
        return tuple(outs)

    devices = jax.devices()[:NCORES]
    mesh = Mesh(np.asarray(devices), ("core",))
    spec = NamedSharding(mesh, PartitionSpec("core"))
    sharded = jax.jit(
        shard_map(_body, mesh=mesh,
                  in_specs=(PartitionSpec("core"),) * (n_params + len(out_names)),
                  out_specs=(PartitionSpec("core"),) * len(out_names),
                  check_rep=False),
        keep_unused=True)
    concat_in = [
        jax.device_put(
            np.concatenate([in_maps[c][n] for c in range(NCORES)], 0), spec)
        for n in in_names]
    concat_zeros = [
        jax.device_put(
            np.zeros((NCORES * a.shape[0], *a.shape[1:]), a.dtype), spec)
        for a in out_avals]

    def run():
        return sharded(*concat_in, *concat_zeros)

    def collect(out_arrs):
        full = np.asarray(out_arrs[0]).reshape(NCORES, L, D)
        out = np.zeros((B, L, D), np.float64)
        for c in range(NCORES):
            out[c // CPB] += full[c]
        return out.astype(np.float32)

    return run, collect
